# revision 1
# baseline (speedup 1.0000x reference)
"""Bass/Trainium2 kernel for nn_BaselineLSTM (B=2048, T=512, H=128, twin=256).

Strategy:
  - Data-parallel: batch 2048 -> 8 cores x 256; each core runs 2 interleaved
    chunks of 128 batch (pipelining hides per-step cross-engine latency).
  - State kept transposed: hT/cT = [H=128 partitions, batch free]. Gate
    matmuls are out[gate_rows, batch] = W_slice.T.T @ hT -> no per-step
    transpose anywhere.
  - Gates land in one PSUM bank per chunk-step ordered [i|f|o|g] so a single
    merged Sigmoid ACT covers i,f,o; Tanh covers g.
  - Phase P (teacher forcing): input + bias enter via a K=2 accumulating
    matmul against packed rows [y_t; 1].
  - Phase H (autoregressive): x_t = W_out h + b_out is folded into the
    recurrence:  g = (W_hh + W_ih W_out) h + (b + W_ih b_out). No feedback
    data path; bias enters via a K=1 matmul against a ones row.
  - h lives in a 4-slot SBUF ring; predictions p_t = W_out h_t are batched
    4 steps per matmul (shifted zero-padded stationary places each group in
    its own PSUM row), flushed to DRAM every 128 steps. b_out is added on
    the host.
  - The two chunks are emitted half a step out of phase (software pipeline);
    tanh(g) is issued before the i/f/o sigmoid so the c-update chain starts
    as early as possible. All matmul/elementwise data is bf16 (PSUM
    accumulation in f32); rel err vs the f32 reference is ~0.9% of absmax.
"""

import functools

import ml_dtypes
import numpy as np

import concourse.bacc as bacc
import concourse.tile as tile
from concourse import mybir
from concourse.bass_utils import run_bass_kernel_spmd

F32 = mybir.dt.float32
BF16 = mybir.dt.bfloat16
AF = mybir.ActivationFunctionType

H = 128          # hidden
NCORES = 8
BS = 256         # batch per core
BC = 128         # batch per chunk
NCHUNK = 2

# pytorch gate order (i, f, g, o) -> kernel order (i, f, o, g)
_PERM = np.concatenate([np.arange(0, 128), np.arange(128, 256),
                        np.arange(384, 512), np.arange(256, 384)])


def _build_body(tc, d, NP, NH, EPOCH):
    nc = tc.nc
    NT = NP + NH

    import contextlib
    with contextlib.ExitStack() as ctx:
        consts = ctx.enter_context(tc.tile_pool(name="consts", bufs=1))
        state = ctx.enter_context(tc.tile_pool(name="state", bufs=1))
        work = ctx.enter_context(tc.tile_pool(name="work", bufs=3))
        gpool = ctx.enter_context(tc.tile_pool(name="gates", bufs=3, space="PSUM"))
        ppool = ctx.enter_context(tc.tile_pool(name="ppsum", bufs=1, space="PSUM"))

        # ---- constants to SBUF
        whhT_p = consts.tile([H, 4 * H], BF16, tag="whhT_p")
        whhT_h = consts.tile([H, 4 * H], BF16, tag="whhT_h")
        lp = consts.tile([2, 4 * H], BF16, tag="lp")
        lh = consts.tile([1, 4 * H], BF16, tag="lh")
        woutZ = consts.tile([H, 2 * H], BF16, tag="woutZ")
        xq = consts.tile([2, NP * BS], BF16, tag="xq")
        ones = consts.tile([1, BS], BF16, tag="ones")
        nc.vector.memset(ones, 1.0)
        nc.sync.dma_start(out=whhT_p, in_=d["whhT_p"])
        nc.sync.dma_start(out=whhT_h, in_=d["whhT_h"])
        nc.sync.dma_start(out=lp, in_=d["lp"])
        nc.sync.dma_start(out=lh, in_=d["lh"])
        nc.sync.dma_start(out=woutZ, in_=d["woutZ"])
        nc.sync.dma_start(out=xq, in_=d["xq"])

        # ---- state: h kept in a 4-slot ring (slot s%4) so predictions can
        # be batched 4 steps per matmul against consecutive slots.
        hist = []
        cT = []
        for ch in range(NCHUNK):
            hh = state.tile([H, 4 * BC], BF16, tag=f"hist{ch}")
            c = state.tile([H, BC], BF16, tag=f"cT{ch}")
            nc.vector.memset(hh, 0.0)
            nc.vector.memset(c, 0.0)
            hist.append(hh)
            cT.append(c)

        pps = [None, None]
        sigs = [None, None]
        tgs = [None, None]
        gates_l = [None, None]

        def front(s, ch):
            """Gate matmuls + sigmoid/tanh activations for step s."""
            phase_p = s < NP
            gates = gpool.tile([H, 4 * H], F32, tag=f"g{ch}",
                               name=f"g{ch}_{s}")
            gates_l[ch] = gates
            whh = whhT_p if phase_p else whhT_h
            hprev = hist[ch][:, ((s - 1) % 4) * BC: ((s - 1) % 4 + 1) * BC]

            def gate_mm(k):
                go = gates[:, k * H:(k + 1) * H]
                nc.tensor.matmul(go, whh[:, k * H:(k + 1) * H], hprev,
                                 start=True, stop=False)
                if phase_p:
                    rhs = xq[0:2, s * BS + ch * BC: s * BS + ch * BC + BC]
                    lhs2 = lp[0:2, k * H:(k + 1) * H]
                else:
                    rhs = ones[0:1, ch * BC: ch * BC + BC]
                    lhs2 = lh[0:1, k * H:(k + 1) * H]
                nc.tensor.matmul(go, lhs2, rhs, start=False, stop=True)

            # g-gate first so tanh(g) can run on ACT while i/f/o matmuls
            # are still streaming; sigmoid follows.
            gate_mm(3)
            tg = work.tile([H, BC], BF16, tag=f"tg{ch}", name=f"tg{ch}_{s}")
            nc.scalar.activation(tg, gates[:, 3 * H:4 * H], AF.Tanh)
            for k in (0, 1, 2):
                gate_mm(k)
            sig = work.tile([H, 3 * H], BF16, tag=f"sig{ch}",
                            name=f"sig{ch}_{s}")
            nc.scalar.activation(sig, gates[:, 0:3 * H], AF.Sigmoid)
            sigs[ch] = sig
            tgs[ch] = tg

        def back(s, ch):
            """c/h update for step s + batched prediction matmul."""
            sig, tg = sigs[ch], tgs[ch]
            t2 = work.tile([H, BC], BF16, tag=f"t2{ch}", name=f"t2{ch}_{s}")
            nc.vector.tensor_mul(t2, sig[:, H:2 * H], cT[ch])
            t1 = work.tile([H, BC], BF16, tag=f"t1{ch}", name=f"t1{ch}_{s}")
            nc.gpsimd.tensor_mul(t1, sig[:, 0:H], tg)
            nc.vector.tensor_add(cT[ch], t2, t1)
            tcn = work.tile([H, BC], BF16, tag=f"tcn{ch}", name=f"tcn{ch}_{s}")
            nc.scalar.activation(tcn, cT[ch], AF.Tanh)
            hslot = hist[ch][:, (s % 4) * BC: (s % 4 + 1) * BC]
            nc.vector.tensor_mul(hslot, sig[:, 2 * H:3 * H], tcn)

            # Predictions: every 4 steps, p for steps 4G..4G+3 = one matmul
            # W_out @ [h_0|h_1|h_2|h_3]; row placement via shifted zero-pad.
            if s % 4 == 3 or s == NT - 1:
                G = s // 4
                r = G % 32
                n = (s % 4 + 1) * BC
                if r == 0:
                    pps[ch] = ppool.tile([H, 4 * BC], F32, tag=f"pps{ch}",
                                         name=f"pps{ch}_{s}")
                nc.tensor.matmul(pps[ch][:, 0:n],
                                 woutZ[:, H - r: 2 * H - r],
                                 hist[ch][:, 0:n],
                                 start=(r == 0), stop=(r == 31 or s == NT - 1),
                                 skip_group_check=True)
                if r == 31 or s == NT - 1:
                    e = G // 32
                    pc = work.tile([32, 4 * BC], F32, tag=f"pc{ch}",
                                   name=f"pc{ch}_{s}")
                    nc.vector.tensor_copy(pc, pps[ch][0:32, :])
                    nc.sync.dma_start(out=d["preds"][e, ch], in_=pc)

        # Software pipeline: chunk 1 runs half a step behind chunk 0 so
        # engines ping-pong between the two independent recurrences.
        for s in range(NT):
            front(s, 0)
            if s > 0:
                back(s - 1, 1)
            front(s, 1)
            back(s, 0)
        back(NT - 1, 1)


@functools.lru_cache(maxsize=2)
def _program(NP, NH, EPOCH):
    nc = bacc.Bacc("TRN2", target_bir_lowering=False, debug=False,
                   num_devices=NCORES)
    NT = NP + NH
    NEP = (NT + 127) // 128
    d = {
        "whhT_p": nc.dram_tensor("whhT_p", [H, 4 * H], BF16,
                                 kind="ExternalInput").ap(),
        "whhT_h": nc.dram_tensor("whhT_h", [H, 4 * H], BF16,
                                 kind="ExternalInput").ap(),
        "lp": nc.dram_tensor("lp", [2, 4 * H], BF16, kind="ExternalInput").ap(),
        "lh": nc.dram_tensor("lh", [1, 4 * H], BF16, kind="ExternalInput").ap(),
        "woutZ": nc.dram_tensor("woutZ", [H, 2 * H], BF16,
                                kind="ExternalInput").ap(),
        "xq": nc.dram_tensor("xq", [2, NP * BS], BF16,
                             kind="ExternalInput").ap(),
        "preds": nc.dram_tensor("preds", [NEP, NCHUNK, 32, 4 * BC], F32,
                                kind="ExternalOutput").ap(),
    }
    with tile.TileContext(nc) as tc:
        _build_body(tc, d, NP, NH, EPOCH)
    nc.compile()
    return nc


def _host_prep(y_flow, W_ih, W_hh, b_ih, b_hh, W_out, b_out, NP):
    """Build per-core input maps. y_flow: (B, T, 1) f32."""
    bf = ml_dtypes.bfloat16
    W_ih = np.asarray(W_ih, np.float32)
    W_hh = np.asarray(W_hh, np.float32)
    W_out = np.asarray(W_out, np.float32)
    bias = np.asarray(b_ih, np.float32) + np.asarray(b_hh, np.float32)
    b_out = np.asarray(b_out, np.float32)

    W_hh_H = W_hh + W_ih @ W_out          # [4H, H]
    bias_H = bias + W_ih[:, 0] * b_out[0]

    whhT_p = np.ascontiguousarray(W_hh[_PERM].T).astype(bf)      # [H, 4H]
    whhT_h = np.ascontiguousarray(W_hh_H[_PERM].T).astype(bf)
    lp = np.stack([W_ih[_PERM, 0], bias[_PERM]]).astype(bf)       # [2, 4H]
    lh = bias_H[_PERM][None, :].astype(bf)                        # [1, 4H]
    woutZ = np.zeros((H, 2 * H), np.float32)                      # [H, 256]
    woutZ[:, H] = W_out[0]
    woutZ = woutZ.astype(bf)

    y = np.asarray(y_flow, np.float32)[:, :, 0]                   # [B, T]
    B = y.shape[0]
    in_maps = []
    for core in range(NCORES):
        yc = y[core * BS:(core + 1) * BS]                         # [BS, T]
        xq = np.ones((2, NP * BS), np.float32)
        xq[0] = yc[:, :NP].T.reshape(-1)
        in_maps.append({
            "whhT_p": whhT_p, "whhT_h": whhT_h, "lp": lp, "lh": lh,
            "woutZ": woutZ, "xq": xq.astype(bf),
        })
    return in_maps


def kernel(y_flow, x_dyn, W_ih, W_hh, b_ih, b_hh, W_out, b_out, twin_idx,
           _trace=False):
    twin = int(twin_idx)
    assert twin == 256, f"kernel hardcodes twin_idx=256, got {twin}"
    B, T, _ = y_flow.shape
    assert (B, T) == (2048, 512)
    NP, NH, EPOCH = twin - 1, T - twin, 128
    NT = NP + NH

    nc = _program(NP, NH, EPOCH)
    in_maps = _host_prep(y_flow, W_ih, W_hh, b_ih, b_hh, W_out, b_out, NP)
    res = run_bass_kernel_spmd(nc, in_maps, core_ids=list(range(NCORES)),
                               trace=_trace)

    b_out = np.asarray(b_out, np.float32)
    out = np.empty((B, NT, 1), np.float32)
    for core in range(NCORES):
        p = np.asarray(res.results[core]["preds"], np.float32)
        nep = p.shape[0]
        a = p.reshape(nep, NCHUNK, 32, 4, BC)      # [e, ch, r, j, b]
        for ch in range(NCHUNK):
            blk = a[:, ch].transpose(3, 0, 1, 2).reshape(BC, -1)[:, :NT]
            out[core * BS + ch * BC: core * BS + (ch + 1) * BC, :, 0] = \
                blk + b_out[0]
    if _trace:
        kernel._last_results = res
    return out



# revision 4
# speedup vs baseline: 1.7008x; 1.7008x over previous
"""Bass/Trainium2 kernel for nn_BaselineLSTM (B=2048, T=512, H=128, twin=256).

Strategy (v2):
  - Data-parallel: batch 2048 -> 8 cores x 256; each core runs 2 interleaved
    chunks of 128 batch (pipelining hides per-step cross-engine latency).
  - State kept transposed: h/c = [H=128 partitions, batch free]; state
    variables are scaled: hT = h/2, cT = 2c, so that every tanh can be
    computed as a sigmoid and all fix-up constants fold into weights:
      tanh(x) = 2*sigmoid(2x) - 1.
  - ONE sigmoid ACT per chunk-step covers all four gates [i|f|o|g]: the
    g-block rows of the stationary weights are pre-scaled so the matmul
    emits 2*pre_g there; a second small sigmoid covers sigma(cT)=sigma(2c).
  - Input + bias enter via ONE K=8 (phase P) / K=4 (phase H) matmul with a
    block-diagonal rhs (phase P rhs streamed from DRAM, phase H rhs static),
    accumulated into the gates PSUM bank before the 4 recurrent matmuls.
  - Cell update on DVE only (gpsimd is pathologically slow for elementwise):
      t2 = sf*cT;  u = (s2g-0.5)*si;  cT = 4u + t2       (scalar_tensor_tensor)
      hT = (sigma(cT)-0.5)*so                             (scalar_tensor_tensor)
  - fp16 everywhere on-chip (not bf16): the 2*sigmoid(2x)-1 rewrite loses
    absolute precision near 0.5 in bf16; fp16's 10 mantissa bits restore it,
    and fp16 keeps the DVE 2x/4x packed modes.
  - Predictions p_t = (2*W_out) hT_t (+ b_out on host): hT kept in a 4-slot
    ring; one shifted-stationary matmul per 4 steps accumulates 128 steps
    into one PSUM bank, flushed to DRAM per 128-step epoch.
"""

import functools

import numpy as np

import concourse.bacc as bacc
import concourse.tile as tile
from concourse import mybir
from concourse.bass_utils import run_bass_kernel_spmd

F32 = mybir.dt.float32
FP16 = mybir.dt.float16
AF = mybir.ActivationFunctionType
OP = mybir.AluOpType

H = 128          # hidden
NCORES = 8
BS = 256         # batch per core
BC = 128         # batch per chunk
NCHUNK = 2
BLK = 32         # xq steps per DMA block

# pytorch gate order (i, f, g, o) -> kernel order (i, f, o, g)
_PERM = np.concatenate([np.arange(0, 128), np.arange(128, 256),
                        np.arange(384, 512), np.arange(256, 384)])
# pre-scales for recurrent weights: x2 compensates hT=h/2; g doubled again
# so the matmul emits 2*pre_g for the tanh->sigmoid rewrite.
_SCALE = np.repeat([2.0, 2.0, 2.0, 4.0], 128)
# pre-scales for input/bias terms: no hT compensation, only the g doubling.
_SCALE_B = np.repeat([1.0, 1.0, 1.0, 2.0], 128)


def _build_body(tc, d, NP, NH):
    nc = tc.nc
    NT = NP + NH
    NBLK = (NP + BLK - 1) // BLK

    import contextlib
    with contextlib.ExitStack() as ctx:
        consts = ctx.enter_context(tc.tile_pool(name="consts", bufs=1))
        state = ctx.enter_context(tc.tile_pool(name="state", bufs=1))
        spool = ctx.enter_context(tc.tile_pool(name="sig", bufs=3))
        wpool = ctx.enter_context(tc.tile_pool(name="work", bufs=3))
        xpool = ctx.enter_context(tc.tile_pool(name="xq", bufs=2))
        gpool = ctx.enter_context(tc.tile_pool(name="gates", bufs=2, space="PSUM"))
        ppool = ctx.enter_context(tc.tile_pool(name="ppsum", bufs=1, space="PSUM"))

        # ---- constants to SBUF
        whhT_p = consts.tile([H, 4 * H], FP16, tag="whhT_p")
        whhT_h = consts.tile([H, 4 * H], FP16, tag="whhT_h")
        bp8 = consts.tile([8, H], FP16, tag="bp8")
        bh4 = consts.tile([4, H], FP16, tag="bh4")
        ones4 = consts.tile([4, 4 * BC], FP16, tag="ones4")
        woutZ = consts.tile([H, 2 * H], FP16, tag="woutZ")
        nc.sync.dma_start(out=whhT_p, in_=d["whhT_p"])
        nc.sync.dma_start(out=whhT_h, in_=d["whhT_h"])
        nc.sync.dma_start(out=bp8, in_=d["bp8"])
        nc.sync.dma_start(out=bh4, in_=d["bh4"])
        nc.sync.dma_start(out=ones4, in_=d["ones4"])
        nc.sync.dma_start(out=woutZ, in_=d["woutZ"])

        # ---- state
        hist = []
        cT = []
        for ch in range(NCHUNK):
            hh = state.tile([H, 4 * BC], FP16, tag=f"hist{ch}")
            c = state.tile([H, BC], FP16, tag=f"cT{ch}")
            nc.vector.memset(hh, 0.0)
            nc.vector.memset(c, 0.0)
            hist.append(hh)
            cT.append(c)

        # ---- xq stream (phase P block-diag rhs), double buffered
        xtiles = [[None] * NBLK for _ in range(NCHUNK)]

        def fetch(blk):
            for ch in range(NCHUNK):
                xt = xpool.tile([8, BLK * 4 * BC], FP16, tag=f"xq{ch}",
                                name=f"xq{ch}_{blk}")
                nc.sync.dma_start(out=xt, in_=d["xq"][ch, blk])
                xtiles[ch][blk] = xt

        fetch(0)
        fetch(1)

        s4s = [None, None]
        pps = [None, None]

        def front(s, ch):
            """input/bias MM + 4 gate MMs + one merged sigmoid."""
            phase_p = s < NP
            gates = gpool.tile([H, 4 * BC], F32, tag=f"g{ch}",
                               name=f"g{ch}_{s}")
            if phase_p:
                blk, sl = divmod(s, BLK)
                rhs = xtiles[ch][blk][:, sl * 4 * BC:(sl + 1) * 4 * BC]
                nc.tensor.matmul(gates, bp8, rhs, start=True, stop=False,
                                 skip_group_check=True)
            else:
                nc.tensor.matmul(gates, bh4, ones4, start=True, stop=False,
                                 skip_group_check=True)
            whh = whhT_p if phase_p else whhT_h
            hprev = hist[ch][:, ((s - 1) % 4) * BC: ((s - 1) % 4 + 1) * BC]
            for j in range(4):
                nc.tensor.matmul(gates[:, j * H:(j + 1) * H],
                                 whh[:, j * H:(j + 1) * H], hprev,
                                 start=False, stop=(j == 3),
                                 skip_group_check=True)
            s4 = spool.tile([H, 4 * BC], FP16, tag=f"s4{ch}",
                            name=f"s4{ch}_{s}")
            nc.scalar.activation(s4, gates, AF.Sigmoid)
            s4s[ch] = s4

        def back(s, ch):
            """cell update on DVE + sigma(cT) + hT + batched prediction MM."""
            s4 = s4s[ch]
            t2 = wpool.tile([H, BC], FP16, tag=f"t2{ch}", name=f"t2{ch}_{s}")
            nc.vector.tensor_mul(t2, s4[:, H:2 * H], cT[ch])
            u = wpool.tile([H, BC], FP16, tag=f"u{ch}", name=f"u{ch}_{s}")
            nc.vector.scalar_tensor_tensor(u, s4[:, 3 * H:4 * H], 0.5,
                                           s4[:, 0:H], OP.subtract, OP.mult)
            nc.vector.scalar_tensor_tensor(cT[ch], u, 4.0, t2,
                                           OP.mult, OP.add)
            sc = wpool.tile([H, BC], FP16, tag=f"sc{ch}", name=f"sc{ch}_{s}")
            nc.scalar.activation(sc, cT[ch], AF.Sigmoid)
            hslot = hist[ch][:, (s % 4) * BC: (s % 4 + 1) * BC]
            nc.vector.scalar_tensor_tensor(hslot, sc, 0.5, s4[:, 2 * H:3 * H],
                                           OP.subtract, OP.mult)

            # Predictions: every 4 steps, one matmul W_out @ [h0|h1|h2|h3];
            # row placement via shifted zero-padded stationary.
            if s % 4 == 3 or s == NT - 1:
                G = s // 4
                r = G % 32
                n = (s % 4 + 1) * BC
                if r == 0:
                    pps[ch] = ppool.tile([H, 4 * BC], F32, tag=f"pps{ch}",
                                         name=f"pps{ch}_{s}")
                nc.tensor.matmul(pps[ch][:, 0:n],
                                 woutZ[:, H - r: 2 * H - r],
                                 hist[ch][:, 0:n],
                                 start=(r == 0), stop=(r == 31 or s == NT - 1),
                                 skip_group_check=True)
                if r == 31 or s == NT - 1:
                    e = G // 32
                    pc = wpool.tile([32, 4 * BC], F32, tag=f"pc{ch}",
                                    name=f"pc{ch}_{s}")
                    nc.vector.tensor_copy(pc, pps[ch][0:32, :])
                    nc.sync.dma_start(out=d["preds"][e, ch], in_=pc)

        # Software pipeline: chunk 1 runs half a step behind chunk 0.
        for s in range(NT):
            if s % BLK == BLK // 2:
                nb = s // BLK + 2
                if nb < NBLK:
                    fetch(nb)
            if s > 0:
                back(s - 1, 1)
            front(s, 0)
            front(s, 1)
            back(s, 0)
        back(NT - 1, 1)


@functools.lru_cache(maxsize=2)
def _program(NP, NH):
    nc = bacc.Bacc("TRN2", target_bir_lowering=False, debug=False,
                   num_devices=NCORES)
    NT = NP + NH
    NEP = (NT + 127) // 128
    NBLK = (NP + BLK - 1) // BLK
    d = {
        "whhT_p": nc.dram_tensor("whhT_p", [H, 4 * H], FP16,
                                 kind="ExternalInput").ap(),
        "whhT_h": nc.dram_tensor("whhT_h", [H, 4 * H], FP16,
                                 kind="ExternalInput").ap(),
        "bp8": nc.dram_tensor("bp8", [8, H], FP16, kind="ExternalInput").ap(),
        "bh4": nc.dram_tensor("bh4", [4, H], FP16, kind="ExternalInput").ap(),
        "ones4": nc.dram_tensor("ones4", [4, 4 * BC], FP16,
                                kind="ExternalInput").ap(),
        "woutZ": nc.dram_tensor("woutZ", [H, 2 * H], FP16,
                                kind="ExternalInput").ap(),
        "xq": nc.dram_tensor("xq", [NCHUNK, NBLK, 8, BLK * 4 * BC], FP16,
                             kind="ExternalInput").ap(),
        "preds": nc.dram_tensor("preds", [NEP, NCHUNK, 32, 4 * BC], F32,
                                kind="ExternalOutput").ap(),
    }
    with tile.TileContext(nc) as tc:
        _build_body(tc, d, NP, NH)
    nc.compile()
    return nc


def _host_prep(y_flow, W_ih, W_hh, b_ih, b_hh, W_out, b_out, NP):
    """Build per-core input maps. y_flow: (B, T, 1) f32."""
    f16 = np.float16
    W_ih = np.asarray(W_ih, np.float32)
    W_hh = np.asarray(W_hh, np.float32)
    W_out = np.asarray(W_out, np.float32)
    bias = np.asarray(b_ih, np.float32) + np.asarray(b_hh, np.float32)
    b_out = np.asarray(b_out, np.float32)

    W_eff = W_hh + W_ih @ W_out           # [4H, H] (phase-H feedback fold)
    b_eff = bias + W_ih[:, 0] * b_out[0]

    sc = _SCALE[:, None]
    whhT_p = np.ascontiguousarray((W_hh[_PERM] * sc).T).astype(f16)
    whhT_h = np.ascontiguousarray((W_eff[_PERM] * sc).T).astype(f16)

    wih_s = (W_ih[_PERM, 0] * _SCALE_B).astype(np.float32)
    b_s = (bias[_PERM] * _SCALE_B).astype(np.float32)
    beff_s = (b_eff[_PERM] * _SCALE_B).astype(np.float32)

    bp8 = np.zeros((8, H), np.float32)
    bh4 = np.zeros((4, H), np.float32)
    ones4 = np.zeros((4, 4 * BC), np.float32)
    for j in range(4):
        bp8[2 * j] = wih_s[j * H:(j + 1) * H]
        bp8[2 * j + 1] = b_s[j * H:(j + 1) * H]
        bh4[j] = beff_s[j * H:(j + 1) * H]
        ones4[j, j * BC:(j + 1) * BC] = 1.0

    woutZ = np.zeros((H, 2 * H), np.float32)
    woutZ[:, H] = 2.0 * W_out[0]

    NBLK = (NP + BLK - 1) // BLK
    NPAD = NBLK * BLK
    y = np.asarray(y_flow, np.float32)[:, :, 0]                   # [B, T]
    in_maps = []
    for core in range(NCORES):
        yc = y[core * BS:(core + 1) * BS]                         # [BS, T]
        xq = np.zeros((NCHUNK, NPAD, 8, 4 * BC), np.float32)
        for ch in range(NCHUNK):
            ystep = yc[ch * BC:(ch + 1) * BC, :NP].T              # [NP, BC]
            for j in range(4):
                xq[ch, :NP, 2 * j, j * BC:(j + 1) * BC] = ystep
                xq[ch, :, 2 * j + 1, j * BC:(j + 1) * BC] = 1.0
        # [ch, NBLK, BLK, 8, 512] -> [ch, NBLK, 8, BLK*512]
        xq = xq.reshape(NCHUNK, NBLK, BLK, 8, 4 * BC)
        xq = np.ascontiguousarray(xq.transpose(0, 1, 3, 2, 4))
        xq = xq.reshape(NCHUNK, NBLK, 8, BLK * 4 * BC)
        in_maps.append({
            "whhT_p": whhT_p, "whhT_h": whhT_h,
            "bp8": bp8.astype(f16), "bh4": bh4.astype(f16),
            "ones4": ones4.astype(f16), "woutZ": woutZ.astype(f16),
            "xq": xq.astype(f16),
        })
    return in_maps


def kernel(y_flow, x_dyn, W_ih, W_hh, b_ih, b_hh, W_out, b_out, twin_idx,
           _trace=False):
    twin = int(twin_idx)
    assert twin == 256, f"kernel hardcodes twin_idx=256, got {twin}"
    B, T, _ = y_flow.shape
    assert (B, T) == (2048, 512)
    NP, NH = twin - 1, T - twin
    NT = NP + NH

    nc = _program(NP, NH)
    in_maps = _host_prep(y_flow, W_ih, W_hh, b_ih, b_hh, W_out, b_out, NP)
    res = run_bass_kernel_spmd(nc, in_maps, core_ids=list(range(NCORES)),
                               trace=_trace)

    b_out = np.asarray(b_out, np.float32)
    out = np.empty((B, NT, 1), np.float32)
    for core in range(NCORES):
        p = np.asarray(res.results[core]["preds"], np.float32)
        nep = p.shape[0]
        a = p.reshape(nep, NCHUNK, 32, 4, BC)      # [e, ch, r, j, b]
        for ch in range(NCHUNK):
            blk = a[:, ch].transpose(3, 0, 1, 2).reshape(BC, -1)[:, :NT]
            out[core * BS + ch * BC: core * BS + (ch + 1) * BC, :, 0] = \
                blk + b_out[0]
    if _trace:
        kernel._last_results = res
    return out


# revision 8
# speedup vs baseline: 1.7093x; 1.0050x over previous
"""Bass/Trainium2 kernel for nn_BaselineLSTM (B=2048, T=512, H=128, twin=256).

Strategy (v2):
  - Data-parallel: batch 2048 -> 8 cores x 256; each core runs 2 interleaved
    chunks of 128 batch (pipelining hides per-step cross-engine latency).
  - State kept transposed: h/c = [H=128 partitions, batch free]; state
    variables are scaled: hT = h/2, cT = 2c, so that every tanh can be
    computed as a sigmoid and all fix-up constants fold into weights:
      tanh(x) = 2*sigmoid(2x) - 1.
  - ONE sigmoid ACT per chunk-step covers all four gates [i|f|o|g]: the
    g-block rows of the stationary weights are pre-scaled so the matmul
    emits 2*pre_g there; a second small sigmoid covers sigma(cT)=sigma(2c).
  - Input + bias enter via ONE K=8 (phase P) / K=4 (phase H) matmul with a
    block-diagonal rhs (phase P rhs streamed from DRAM, phase H rhs static),
    accumulated into the gates PSUM bank before the 4 recurrent matmuls.
  - Cell update on DVE only (gpsimd is pathologically slow for elementwise):
      t2 = sf*cT;  u = (s2g-0.5)*si;  cT = 4u + t2       (scalar_tensor_tensor)
      hT = (sigma(cT)-0.5)*so                             (scalar_tensor_tensor)
  - fp16 everywhere on-chip (not bf16): the 2*sigmoid(2x)-1 rewrite loses
    absolute precision near 0.5 in bf16; fp16's 10 mantissa bits restore it,
    and fp16 keeps the DVE 2x/4x packed modes.
  - Predictions p_t = (2*W_out) hT_t (+ b_out on host): hT kept in a 4-slot
    ring; one shifted-stationary matmul per 4 steps accumulates 128 steps
    into one PSUM bank, flushed to DRAM per 128-step epoch.
"""

import functools

import numpy as np

import concourse.bacc as bacc
import concourse.tile as tile
from concourse import mybir
from concourse.bass_utils import run_bass_kernel_spmd

F32 = mybir.dt.float32
FP16 = mybir.dt.float16
AF = mybir.ActivationFunctionType
OP = mybir.AluOpType

H = 128          # hidden
NCORES = 8
BS = 256         # batch per core
BC = 128         # batch per chunk
NCHUNK = 2
BLK = 32         # xq steps per DMA block

# pytorch gate order (i, f, g, o) -> kernel order (i, f, o, g)
_PERM = np.concatenate([np.arange(0, 128), np.arange(128, 256),
                        np.arange(384, 512), np.arange(256, 384)])
# pre-scales for recurrent weights: x2 compensates hT=h/2; g doubled again
# so the matmul emits 2*pre_g for the tanh->sigmoid rewrite.
_SCALE = np.repeat([2.0, 2.0, 2.0, 4.0], 128)
# pre-scales for input/bias terms: no hT compensation, only the g doubling.
_SCALE_B = np.repeat([1.0, 1.0, 1.0, 2.0], 128)


def _build_body(tc, d, NP, NH):
    nc = tc.nc
    NT = NP + NH
    NBLK = (NP + BLK - 1) // BLK

    import contextlib
    with contextlib.ExitStack() as ctx:
        consts = ctx.enter_context(tc.tile_pool(name="consts", bufs=1))
        state = ctx.enter_context(tc.tile_pool(name="state", bufs=1))
        spool = ctx.enter_context(tc.tile_pool(name="sig", bufs=3))
        wpool = ctx.enter_context(tc.tile_pool(name="work", bufs=3))
        xpool = ctx.enter_context(tc.tile_pool(name="xq", bufs=2))
        gpool = ctx.enter_context(tc.tile_pool(name="gates", bufs=2, space="PSUM"))
        ppool = ctx.enter_context(tc.tile_pool(name="ppsum", bufs=1, space="PSUM"))
        wupool = ctx.enter_context(tc.tile_pool(name="warmps", bufs=1, space="PSUM"))

        # ---- constants to SBUF
        whhT_p = consts.tile([H, 4 * H], FP16, tag="whhT_p")
        whhT_h = consts.tile([H, 4 * H], FP16, tag="whhT_h")
        bp8 = consts.tile([8, H], FP16, tag="bp8")
        bh4 = consts.tile([4, H], FP16, tag="bh4")
        ones4 = consts.tile([4, 4 * BC], FP16, tag="ones4")
        woutZ = consts.tile([H, 2 * H], FP16, tag="woutZ")
        nc.sync.dma_start(out=whhT_p, in_=d["whhT_p"])
        nc.sync.dma_start(out=whhT_h, in_=d["whhT_h"])
        nc.sync.dma_start(out=bp8, in_=d["bp8"])
        nc.sync.dma_start(out=bh4, in_=d["bh4"])
        nc.sync.dma_start(out=ones4, in_=d["ones4"])
        nc.sync.dma_start(out=woutZ, in_=d["woutZ"])

        # ---- PE warmup: ~7us of dense back-to-back matmuls so the HAM
        # clock-gate releases (PE runs at 2.4 GHz instead of 1.2 GHz for
        # the whole recurrence). Output is read + DMA'd so DCE keeps it.
        wscr = wupool.tile([H, 4 * H], F32, tag="warm")
        for _ in range(16):
            nc.tensor.matmul(wscr, whhT_p[:, 0:H], whhT_p,
                             start=True, stop=True, skip_group_check=True)
        wout = consts.tile([1, 4], F32, tag="wout")
        nc.vector.tensor_copy(wout, wscr[0:1, 0:4])
        nc.sync.dma_start(out=d["warm"], in_=wout)

        # ---- state
        hist = []
        cT = []
        for ch in range(NCHUNK):
            hh = state.tile([H, 4 * BC], FP16, tag=f"hist{ch}")
            c = state.tile([H, BC], FP16, tag=f"cT{ch}")
            nc.vector.memset(hh, 0.0)
            nc.vector.memset(c, 0.0)
            hist.append(hh)
            cT.append(c)

        # ---- xq stream (phase P block-diag rhs), double buffered
        xtiles = [[None] * NBLK for _ in range(NCHUNK)]

        def fetch(blk):
            for ch in range(NCHUNK):
                xt = xpool.tile([8, BLK * 4 * BC], FP16, tag=f"xq{ch}",
                                name=f"xq{ch}_{blk}")
                nc.sync.dma_start(out=xt, in_=d["xq"][ch, blk])
                xtiles[ch][blk] = xt

        fetch(0)
        fetch(1)

        s4s = [None, None]
        pps = [None, None]

        def front(s, ch):
            """input/bias MM + 4 gate MMs + one merged sigmoid."""
            phase_p = s < NP
            gates = gpool.tile([H, 4 * BC], F32, tag=f"g{ch}",
                               name=f"g{ch}_{s}")
            if phase_p:
                blk, sl = divmod(s, BLK)
                rhs = xtiles[ch][blk][:, sl * 4 * BC:(sl + 1) * 4 * BC]
                nc.tensor.matmul(gates, bp8, rhs, start=True, stop=False,
                                 skip_group_check=True)
            else:
                nc.tensor.matmul(gates, bh4, ones4, start=True, stop=False,
                                 skip_group_check=True)
            whh = whhT_p if phase_p else whhT_h
            hprev = hist[ch][:, ((s - 1) % 4) * BC: ((s - 1) % 4 + 1) * BC]
            for j in range(4):
                nc.tensor.matmul(gates[:, j * H:(j + 1) * H],
                                 whh[:, j * H:(j + 1) * H], hprev,
                                 start=False, stop=(j == 3),
                                 skip_group_check=True)
            s4 = spool.tile([H, 4 * BC], FP16, tag=f"s4{ch}",
                            name=f"s4{ch}_{s}")
            nc.scalar.activation(s4, gates, AF.Sigmoid)
            s4s[ch] = s4

        def back(s, ch):
            """cell update on DVE + sigma(cT) + hT + batched prediction MM."""
            s4 = s4s[ch]
            t2 = wpool.tile([H, BC], FP16, tag=f"t2{ch}", name=f"t2{ch}_{s}")
            nc.vector.tensor_mul(t2, s4[:, H:2 * H], cT[ch])
            u = wpool.tile([H, BC], FP16, tag=f"u{ch}", name=f"u{ch}_{s}")
            nc.vector.scalar_tensor_tensor(u, s4[:, 3 * H:4 * H], 0.5,
                                           s4[:, 0:H], OP.subtract, OP.mult)
            nc.vector.scalar_tensor_tensor(cT[ch], u, 4.0, t2,
                                           OP.mult, OP.add)
            sc = wpool.tile([H, BC], FP16, tag=f"sc{ch}", name=f"sc{ch}_{s}")
            nc.scalar.activation(sc, cT[ch], AF.Sigmoid)
            hslot = hist[ch][:, (s % 4) * BC: (s % 4 + 1) * BC]
            nc.vector.scalar_tensor_tensor(hslot, sc, 0.5, s4[:, 2 * H:3 * H],
                                           OP.subtract, OP.mult)

            # Predictions: every 4 steps, one matmul W_out @ [h0|h1|h2|h3];
            # row placement via shifted zero-padded stationary.
            if s % 4 == 3 or s == NT - 1:
                G = s // 4
                r = G % 32
                n = (s % 4 + 1) * BC
                if r == 0:
                    pps[ch] = ppool.tile([H, 4 * BC], F32, tag=f"pps{ch}",
                                         name=f"pps{ch}_{s}")
                nc.tensor.matmul(pps[ch][:, 0:n],
                                 woutZ[:, H - r: 2 * H - r],
                                 hist[ch][:, 0:n],
                                 start=(r == 0), stop=(r == 31 or s == NT - 1),
                                 skip_group_check=True)
                if r == 31 or s == NT - 1:
                    e = G // 32
                    pc = wpool.tile([32, 4 * BC], F32, tag=f"pc{ch}",
                                    name=f"pc{ch}_{s}")
                    nc.vector.tensor_copy(pc, pps[ch][0:32, :])
                    nc.sync.dma_start(out=d["preds"][e, ch], in_=pc)

        # Software pipeline: full A-step then full B-step per iteration.
        # Each engine's FIFO then alternates A-stage / B-stage, which locks
        # the two chunks half a step out of phase (emitting both fronts
        # together lets the chunks drift in-phase and exposes the full
        # serial chain latency).
        for s in range(NT):
            if s % BLK == BLK // 2:
                nb = s // BLK + 2
                if nb < NBLK:
                    fetch(nb)
            front(s, 0)
            back(s, 0)
            front(s, 1)
            back(s, 1)


@functools.lru_cache(maxsize=2)
def _program(NP, NH):
    nc = bacc.Bacc("TRN2", target_bir_lowering=False, debug=False,
                   num_devices=NCORES)
    NT = NP + NH
    NEP = (NT + 127) // 128
    NBLK = (NP + BLK - 1) // BLK
    d = {
        "whhT_p": nc.dram_tensor("whhT_p", [H, 4 * H], FP16,
                                 kind="ExternalInput").ap(),
        "whhT_h": nc.dram_tensor("whhT_h", [H, 4 * H], FP16,
                                 kind="ExternalInput").ap(),
        "bp8": nc.dram_tensor("bp8", [8, H], FP16, kind="ExternalInput").ap(),
        "bh4": nc.dram_tensor("bh4", [4, H], FP16, kind="ExternalInput").ap(),
        "ones4": nc.dram_tensor("ones4", [4, 4 * BC], FP16,
                                kind="ExternalInput").ap(),
        "woutZ": nc.dram_tensor("woutZ", [H, 2 * H], FP16,
                                kind="ExternalInput").ap(),
        "xq": nc.dram_tensor("xq", [NCHUNK, NBLK, 8, BLK * 4 * BC], FP16,
                             kind="ExternalInput").ap(),
        "preds": nc.dram_tensor("preds", [NEP, NCHUNK, 32, 4 * BC], F32,
                                kind="ExternalOutput").ap(),
        "warm": nc.dram_tensor("warm", [1, 4], F32,
                               kind="ExternalOutput").ap(),
    }
    with tile.TileContext(nc) as tc:
        _build_body(tc, d, NP, NH)
    nc.compile()
    return nc


def _host_prep(y_flow, W_ih, W_hh, b_ih, b_hh, W_out, b_out, NP):
    """Build per-core input maps. y_flow: (B, T, 1) f32."""
    f16 = np.float16
    W_ih = np.asarray(W_ih, np.float32)
    W_hh = np.asarray(W_hh, np.float32)
    W_out = np.asarray(W_out, np.float32)
    bias = np.asarray(b_ih, np.float32) + np.asarray(b_hh, np.float32)
    b_out = np.asarray(b_out, np.float32)

    W_eff = W_hh + W_ih @ W_out           # [4H, H] (phase-H feedback fold)
    b_eff = bias + W_ih[:, 0] * b_out[0]

    sc = _SCALE[:, None]
    whhT_p = np.ascontiguousarray((W_hh[_PERM] * sc).T).astype(f16)
    whhT_h = np.ascontiguousarray((W_eff[_PERM] * sc).T).astype(f16)

    wih_s = (W_ih[_PERM, 0] * _SCALE_B).astype(np.float32)
    b_s = (bias[_PERM] * _SCALE_B).astype(np.float32)
    beff_s = (b_eff[_PERM] * _SCALE_B).astype(np.float32)

    bp8 = np.zeros((8, H), np.float32)
    bh4 = np.zeros((4, H), np.float32)
    ones4 = np.zeros((4, 4 * BC), np.float32)
    for j in range(4):
        bp8[2 * j] = wih_s[j * H:(j + 1) * H]
        bp8[2 * j + 1] = b_s[j * H:(j + 1) * H]
        bh4[j] = beff_s[j * H:(j + 1) * H]
        ones4[j, j * BC:(j + 1) * BC] = 1.0

    woutZ = np.zeros((H, 2 * H), np.float32)
    woutZ[:, H] = 2.0 * W_out[0]

    NBLK = (NP + BLK - 1) // BLK
    NPAD = NBLK * BLK
    y = np.asarray(y_flow, np.float32)[:, :, 0]                   # [B, T]
    in_maps = []
    for core in range(NCORES):
        yc = y[core * BS:(core + 1) * BS]                         # [BS, T]
        xq = np.zeros((NCHUNK, NPAD, 8, 4 * BC), np.float32)
        for ch in range(NCHUNK):
            ystep = yc[ch * BC:(ch + 1) * BC, :NP].T              # [NP, BC]
            for j in range(4):
                xq[ch, :NP, 2 * j, j * BC:(j + 1) * BC] = ystep
                xq[ch, :, 2 * j + 1, j * BC:(j + 1) * BC] = 1.0
        # [ch, NBLK, BLK, 8, 512] -> [ch, NBLK, 8, BLK*512]
        xq = xq.reshape(NCHUNK, NBLK, BLK, 8, 4 * BC)
        xq = np.ascontiguousarray(xq.transpose(0, 1, 3, 2, 4))
        xq = xq.reshape(NCHUNK, NBLK, 8, BLK * 4 * BC)
        in_maps.append({
            "whhT_p": whhT_p, "whhT_h": whhT_h,
            "bp8": bp8.astype(f16), "bh4": bh4.astype(f16),
            "ones4": ones4.astype(f16), "woutZ": woutZ.astype(f16),
            "xq": xq.astype(f16),
        })
    return in_maps


def kernel(y_flow, x_dyn, W_ih, W_hh, b_ih, b_hh, W_out, b_out, twin_idx,
           _trace=False):
    twin = int(twin_idx)
    assert twin == 256, f"kernel hardcodes twin_idx=256, got {twin}"
    B, T, _ = y_flow.shape
    assert (B, T) == (2048, 512)
    NP, NH = twin - 1, T - twin
    NT = NP + NH

    nc = _program(NP, NH)
    in_maps = _host_prep(y_flow, W_ih, W_hh, b_ih, b_hh, W_out, b_out, NP)
    res = run_bass_kernel_spmd(nc, in_maps, core_ids=list(range(NCORES)),
                               trace=_trace)

    b_out = np.asarray(b_out, np.float32)
    out = np.empty((B, NT, 1), np.float32)
    for core in range(NCORES):
        p = np.asarray(res.results[core]["preds"], np.float32)
        nep = p.shape[0]
        a = p.reshape(nep, NCHUNK, 32, 4, BC)      # [e, ch, r, j, b]
        for ch in range(NCHUNK):
            blk = a[:, ch].transpose(3, 0, 1, 2).reshape(BC, -1)[:, :NT]
            out[core * BS + ch * BC: core * BS + (ch + 1) * BC, :, 0] = \
                blk + b_out[0]
    if _trace:
        kernel._last_results = res
    return out


# revision 14
# speedup vs baseline: 1.7362x; 1.0158x over previous
"""Bass/Trainium2 kernel for nn_BaselineLSTM (B=2048, T=512, H=128, twin=256).

Strategy (v2):
  - Data-parallel: batch 2048 -> 8 cores x 256; each core runs 2 interleaved
    chunks of 128 batch (pipelining hides per-step cross-engine latency).
  - State kept transposed: h/c = [H=128 partitions, batch free]; state
    variables are scaled: hT = h/2, cT = 2c, so that every tanh can be
    computed as a sigmoid and all fix-up constants fold into weights:
      tanh(x) = 2*sigmoid(2x) - 1.
  - ONE sigmoid ACT per chunk-step covers all four gates [i|f|o|g]: the
    g-block rows of the stationary weights are pre-scaled so the matmul
    emits 2*pre_g there; a second small sigmoid covers sigma(cT)=sigma(2c).
  - Input + bias enter via ONE K=8 (phase P) / K=4 (phase H) matmul with a
    block-diagonal rhs (phase P rhs streamed from DRAM, phase H rhs static),
    accumulated into the gates PSUM bank before the 4 recurrent matmuls.
  - Cell update on DVE only (gpsimd is pathologically slow for elementwise):
      t2 = sf*cT;  u = (s2g-0.5)*si;  cT = 4u + t2       (scalar_tensor_tensor)
      hT = (sigma(cT)-0.5)*so                             (scalar_tensor_tensor)
  - fp16 everywhere on-chip (not bf16): the 2*sigmoid(2x)-1 rewrite loses
    absolute precision near 0.5 in bf16; fp16's 10 mantissa bits restore it,
    and fp16 keeps the DVE 2x/4x packed modes.
  - Predictions p_t = (2*W_out) hT_t (+ b_out on host): hT kept in a 4-slot
    ring; one shifted-stationary matmul per 4 steps accumulates 128 steps
    into one PSUM bank, flushed to DRAM per 128-step epoch.
"""

import functools

import numpy as np

import concourse.bacc as bacc
import concourse.tile as tile
from concourse import mybir
from concourse.bass_utils import run_bass_kernel_spmd

F32 = mybir.dt.float32
FP16 = mybir.dt.float16
AF = mybir.ActivationFunctionType
OP = mybir.AluOpType

H = 128          # hidden
NCORES = 8
BS = 256         # batch per core
BC = 128         # batch per chunk
NCHUNK = 2
BLK = 32         # xq steps per DMA block

# pytorch gate order (i, f, g, o) -> kernel order (i, f, o, g)
_PERM = np.concatenate([np.arange(0, 128), np.arange(128, 256),
                        np.arange(384, 512), np.arange(256, 384)])
# pre-scales: the g-gate rows are doubled so the matmul emits 2*pre_g for
# the tanh(x) = 2*sigmoid(2x)-1 rewrite; i,f,o unscaled (state h is kept
# unscaled, cT = 2c).
_SCALE = np.repeat([1.0, 1.0, 1.0, 2.0], 128)
_SCALE_B = _SCALE


def _build_body(tc, d, NP, NH):
    nc = tc.nc
    NT = NP + NH
    NBLK = (NP + BLK - 1) // BLK

    import contextlib
    with contextlib.ExitStack() as ctx:
        consts = ctx.enter_context(tc.tile_pool(name="consts", bufs=1))
        state = ctx.enter_context(tc.tile_pool(name="state", bufs=1))
        spool = ctx.enter_context(tc.tile_pool(name="sig", bufs=3))
        wpool = ctx.enter_context(tc.tile_pool(name="work", bufs=3))
        xpool = ctx.enter_context(tc.tile_pool(name="xq", bufs=2))
        gpool = ctx.enter_context(tc.tile_pool(name="gates", bufs=2, space="PSUM"))
        ppool = ctx.enter_context(tc.tile_pool(name="ppsum", bufs=1, space="PSUM"))
        wupool = ctx.enter_context(tc.tile_pool(name="warmps", bufs=1, space="PSUM"))

        # ---- constants to SBUF
        whhT_p = consts.tile([H, 4 * H], FP16, tag="whhT_p")
        whhT_h = consts.tile([H, 4 * H], FP16, tag="whhT_h")
        bp8 = consts.tile([8, H], FP16, tag="bp8")
        bh4 = consts.tile([4, H], FP16, tag="bh4")
        ones4 = consts.tile([4, 4 * BC], FP16, tag="ones4")
        woutZ = consts.tile([H, 2 * H], FP16, tag="woutZ")
        nc.sync.dma_start(out=whhT_p, in_=d["whhT_p"])
        nc.sync.dma_start(out=whhT_h, in_=d["whhT_h"])
        nc.sync.dma_start(out=bp8, in_=d["bp8"])
        nc.sync.dma_start(out=bh4, in_=d["bh4"])
        nc.sync.dma_start(out=ones4, in_=d["ones4"])
        nc.sync.dma_start(out=woutZ, in_=d["woutZ"])

        # ---- PE warmup: ~7us of dense back-to-back matmuls so the HAM
        # clock-gate releases (PE runs at 2.4 GHz instead of 1.2 GHz for
        # the whole recurrence). Output is read + DMA'd so DCE keeps it.
        wscr = wupool.tile([H, 4 * H], F32, tag="warm")
        for _ in range(16):
            nc.tensor.matmul(wscr, whhT_p[:, 0:H], whhT_p,
                             start=True, stop=True, skip_group_check=True)

        # ---- state
        hist = []
        cT = []
        for ch in range(NCHUNK):
            hh = state.tile([H, 4 * BC], FP16, tag=f"hist{ch}")
            c = state.tile([H, BC], FP16, tag=f"cT{ch}")
            nc.vector.memset(hh, 0.0)
            nc.vector.memset(c, 0.0)
            hist.append(hh)
            cT.append(c)

        # ---- xq stream (phase P block-diag rhs), double buffered
        xtiles = [[None] * NBLK for _ in range(NCHUNK)]

        def fetch(blk):
            for ch in range(NCHUNK):
                xt = xpool.tile([8, BLK * 4 * BC], FP16, tag=f"xq{ch}",
                                name=f"xq{ch}_{blk}")
                nc.sync.dma_start(out=xt, in_=d["xq"][ch, blk])
                xtiles[ch][blk] = xt

        fetch(0)
        fetch(1)

        s4s = [None, None]
        pps = [None, None]

        def front(s, ch):
            """input/bias MM + 4 gate MMs + one merged sigmoid."""
            phase_p = s < NP
            gates = gpool.tile([H, 4 * BC], F32, tag=f"g{ch}",
                               name=f"g{ch}_{s}")
            if phase_p:
                blk, sl = divmod(s, BLK)
                rhs = xtiles[ch][blk][:, sl * 4 * BC:(sl + 1) * 4 * BC]
                nc.tensor.matmul(gates, bp8, rhs, start=True, stop=False,
                                 skip_group_check=True)
            else:
                nc.tensor.matmul(gates, bh4, ones4, start=True, stop=False,
                                 skip_group_check=True)
            whh = whhT_p if phase_p else whhT_h
            hprev = hist[ch][:, ((s - 1) % 4) * BC: ((s - 1) % 4 + 1) * BC]
            for j in range(4):
                nc.tensor.matmul(gates[:, j * H:(j + 1) * H],
                                 whh[:, j * H:(j + 1) * H], hprev,
                                 start=False, stop=(j == 3),
                                 skip_group_check=True)
            # keep-warm filler: a dummy wide matmul right after the gate MMs
            # raises PE occupancy so the HAM clock-gate holds 2.4 GHz.
            nc.tensor.matmul(wscr, whhT_p[:, 0:H], whhT_p,
                             start=True, stop=True, skip_group_check=True)
            s4 = spool.tile([H, 4 * BC], FP16, tag=f"s4{ch}",
                            name=f"s4{ch}_{s}")
            nc.scalar.activation(s4, gates, AF.Sigmoid)
            s4s[ch] = s4

        def back(s, ch):
            """cell update on DVE + sigma(cT) + hT + batched prediction MM."""
            s4 = s4s[ch]
            t2 = wpool.tile([H, BC], FP16, tag=f"t2{ch}", name=f"t2{ch}_{s}")
            nc.vector.tensor_mul(t2, s4[:, H:2 * H], cT[ch])
            u = wpool.tile([H, BC], FP16, tag=f"u{ch}", name=f"u{ch}_{s}")
            nc.vector.scalar_tensor_tensor(u, s4[:, 3 * H:4 * H], 0.5,
                                           s4[:, 0:H], OP.subtract, OP.mult)
            nc.vector.scalar_tensor_tensor(cT[ch], u, 4.0, t2,
                                           OP.mult, OP.add)
            tc_ = wpool.tile([H, BC], FP16, tag=f"sc{ch}", name=f"sc{ch}_{s}")
            nc.scalar.activation(tc_, cT[ch], AF.Tanh, scale=0.5)
            hslot = hist[ch][:, (s % 4) * BC: (s % 4 + 1) * BC]
            nc.vector.tensor_mul(hslot, tc_, s4[:, 2 * H:3 * H])

            # Predictions: every 4 steps, one matmul W_out @ [h0|h1|h2|h3];
            # row placement via shifted zero-padded stationary.
            if s % 4 == 3 or s == NT - 1:
                G = s // 4
                r = G % 32
                n = (s % 4 + 1) * BC
                if r == 0:
                    pps[ch] = ppool.tile([H, 4 * BC], F32, tag=f"pps{ch}",
                                         name=f"pps{ch}_{s}")
                nc.tensor.matmul(pps[ch][:, 0:n],
                                 woutZ[:, H - r: 2 * H - r],
                                 hist[ch][:, 0:n],
                                 start=(r == 0), stop=(r == 31 or s == NT - 1),
                                 skip_group_check=True)
                if r == 31 or s == NT - 1:
                    e = G // 32
                    pc = wpool.tile([32, 4 * BC], F32, tag=f"pc{ch}",
                                    name=f"pc{ch}_{s}")
                    nc.vector.tensor_copy(pc, pps[ch][0:32, :])
                    nc.sync.dma_start(out=d["preds"][e, ch], in_=pc)

        # Software pipeline: full A-step then full B-step per iteration.
        # Each engine's FIFO then alternates A-stage / B-stage, which locks
        # the two chunks half a step out of phase (emitting both fronts
        # together lets the chunks drift in-phase and exposes the full
        # serial chain latency).
        for s in range(NT):
            if s % BLK == BLK // 2:
                nb = s // BLK + 2
                if nb < NBLK:
                    fetch(nb)
            front(s, 0)
            back(s, 0)
            front(s, 1)
            back(s, 1)

        # read + DMA the warm-up scratch at the END so DCE keeps every
        # filler matmul above alive.
        wout = consts.tile([1, 4], F32, tag="wout")
        nc.vector.tensor_copy(wout, wscr[0:1, 0:4])
        nc.sync.dma_start(out=d["warm"], in_=wout)


@functools.lru_cache(maxsize=2)
def _program(NP, NH):
    nc = bacc.Bacc("TRN2", target_bir_lowering=False, debug=False,
                   num_devices=NCORES)
    NT = NP + NH
    NEP = (NT + 127) // 128
    NBLK = (NP + BLK - 1) // BLK
    d = {
        "whhT_p": nc.dram_tensor("whhT_p", [H, 4 * H], FP16,
                                 kind="ExternalInput").ap(),
        "whhT_h": nc.dram_tensor("whhT_h", [H, 4 * H], FP16,
                                 kind="ExternalInput").ap(),
        "bp8": nc.dram_tensor("bp8", [8, H], FP16, kind="ExternalInput").ap(),
        "bh4": nc.dram_tensor("bh4", [4, H], FP16, kind="ExternalInput").ap(),
        "ones4": nc.dram_tensor("ones4", [4, 4 * BC], FP16,
                                kind="ExternalInput").ap(),
        "woutZ": nc.dram_tensor("woutZ", [H, 2 * H], FP16,
                                kind="ExternalInput").ap(),
        "xq": nc.dram_tensor("xq", [NCHUNK, NBLK, 8, BLK * 4 * BC], FP16,
                             kind="ExternalInput").ap(),
        "preds": nc.dram_tensor("preds", [NEP, NCHUNK, 32, 4 * BC], F32,
                                kind="ExternalOutput").ap(),
        "warm": nc.dram_tensor("warm", [1, 4], F32,
                               kind="ExternalOutput").ap(),
    }
    with tile.TileContext(nc) as tc:
        _build_body(tc, d, NP, NH)
    nc.compile()
    return nc


def _host_prep(y_flow, W_ih, W_hh, b_ih, b_hh, W_out, b_out, NP):
    """Build per-core input maps. y_flow: (B, T, 1) f32."""
    f16 = np.float16
    W_ih = np.asarray(W_ih, np.float32)
    W_hh = np.asarray(W_hh, np.float32)
    W_out = np.asarray(W_out, np.float32)
    bias = np.asarray(b_ih, np.float32) + np.asarray(b_hh, np.float32)
    b_out = np.asarray(b_out, np.float32)

    W_eff = W_hh + W_ih @ W_out           # [4H, H] (phase-H feedback fold)
    b_eff = bias + W_ih[:, 0] * b_out[0]

    sc = _SCALE[:, None]
    whhT_p = np.ascontiguousarray((W_hh[_PERM] * sc).T).astype(f16)
    whhT_h = np.ascontiguousarray((W_eff[_PERM] * sc).T).astype(f16)

    wih_s = (W_ih[_PERM, 0] * _SCALE_B).astype(np.float32)
    b_s = (bias[_PERM] * _SCALE_B).astype(np.float32)
    beff_s = (b_eff[_PERM] * _SCALE_B).astype(np.float32)

    bp8 = np.zeros((8, H), np.float32)
    bh4 = np.zeros((4, H), np.float32)
    ones4 = np.zeros((4, 4 * BC), np.float32)
    for j in range(4):
        bp8[2 * j] = wih_s[j * H:(j + 1) * H]
        bp8[2 * j + 1] = b_s[j * H:(j + 1) * H]
        bh4[j] = beff_s[j * H:(j + 1) * H]
        ones4[j, j * BC:(j + 1) * BC] = 1.0

    woutZ = np.zeros((H, 2 * H), np.float32)
    woutZ[:, H] = W_out[0]

    NBLK = (NP + BLK - 1) // BLK
    NPAD = NBLK * BLK
    y = np.asarray(y_flow, np.float32)[:, :, 0]                   # [B, T]
    in_maps = []
    for core in range(NCORES):
        yc = y[core * BS:(core + 1) * BS]                         # [BS, T]
        xq = np.zeros((NCHUNK, NPAD, 8, 4 * BC), np.float32)
        for ch in range(NCHUNK):
            ystep = yc[ch * BC:(ch + 1) * BC, :NP].T              # [NP, BC]
            for j in range(4):
                xq[ch, :NP, 2 * j, j * BC:(j + 1) * BC] = ystep
                xq[ch, :, 2 * j + 1, j * BC:(j + 1) * BC] = 1.0
        # [ch, NBLK, BLK, 8, 512] -> [ch, NBLK, 8, BLK*512]
        xq = xq.reshape(NCHUNK, NBLK, BLK, 8, 4 * BC)
        xq = np.ascontiguousarray(xq.transpose(0, 1, 3, 2, 4))
        xq = xq.reshape(NCHUNK, NBLK, 8, BLK * 4 * BC)
        in_maps.append({
            "whhT_p": whhT_p, "whhT_h": whhT_h,
            "bp8": bp8.astype(f16), "bh4": bh4.astype(f16),
            "ones4": ones4.astype(f16), "woutZ": woutZ.astype(f16),
            "xq": xq.astype(f16),
        })
    return in_maps


def kernel(y_flow, x_dyn, W_ih, W_hh, b_ih, b_hh, W_out, b_out, twin_idx,
           _trace=False):
    twin = int(twin_idx)
    assert twin == 256, f"kernel hardcodes twin_idx=256, got {twin}"
    B, T, _ = y_flow.shape
    assert (B, T) == (2048, 512)
    NP, NH = twin - 1, T - twin
    NT = NP + NH

    nc = _program(NP, NH)
    in_maps = _host_prep(y_flow, W_ih, W_hh, b_ih, b_hh, W_out, b_out, NP)
    res = run_bass_kernel_spmd(nc, in_maps, core_ids=list(range(NCORES)),
                               trace=_trace)

    b_out = np.asarray(b_out, np.float32)
    out = np.empty((B, NT, 1), np.float32)
    for core in range(NCORES):
        p = np.asarray(res.results[core]["preds"], np.float32)
        nep = p.shape[0]
        a = p.reshape(nep, NCHUNK, 32, 4, BC)      # [e, ch, r, j, b]
        for ch in range(NCHUNK):
            blk = a[:, ch].transpose(3, 0, 1, 2).reshape(BC, -1)[:, :NT]
            out[core * BS + ch * BC: core * BS + (ch + 1) * BC, :, 0] = \
                blk + b_out[0]
    if _trace:
        kernel._last_results = res
    return out


# revision 23
# speedup vs baseline: 1.7432x; 1.0040x over previous
"""Bass/Trainium2 kernel for nn_BaselineLSTM (B=2048, T=512, H=128, twin=256).

Strategy (v2):
  - Data-parallel: batch 2048 -> 8 cores x 256; each core runs 2 interleaved
    chunks of 128 batch (pipelining hides per-step cross-engine latency).
  - State kept transposed: h/c = [H=128 partitions, batch free]; state
    variables are scaled: hT = h/2, cT = 2c, so that every tanh can be
    computed as a sigmoid and all fix-up constants fold into weights:
      tanh(x) = 2*sigmoid(2x) - 1.
  - ONE sigmoid ACT per chunk-step covers all four gates [i|f|o|g]: the
    g-block rows of the stationary weights are pre-scaled so the matmul
    emits 2*pre_g there; a second small sigmoid covers sigma(cT)=sigma(2c).
  - Input + bias enter via ONE K=8 (phase P) / K=4 (phase H) matmul with a
    block-diagonal rhs (phase P rhs streamed from DRAM, phase H rhs static),
    accumulated into the gates PSUM bank before the 4 recurrent matmuls.
  - Cell update on DVE only (gpsimd is pathologically slow for elementwise):
      t2 = sf*cT;  u = (s2g-0.5)*si;  cT = 4u + t2       (scalar_tensor_tensor)
      hT = (sigma(cT)-0.5)*so                             (scalar_tensor_tensor)
  - fp16 everywhere on-chip (not bf16): the 2*sigmoid(2x)-1 rewrite loses
    absolute precision near 0.5 in bf16; fp16's 10 mantissa bits restore it,
    and fp16 keeps the DVE 2x/4x packed modes.
  - Predictions p_t = (2*W_out) hT_t (+ b_out on host): hT kept in a 4-slot
    ring; one shifted-stationary matmul per 4 steps accumulates 128 steps
    into one PSUM bank, flushed to DRAM per 128-step epoch.
"""

import functools

import numpy as np

import concourse.bacc as bacc
import concourse.tile as tile
from concourse import mybir
from concourse.bass_utils import run_bass_kernel_spmd

F32 = mybir.dt.float32
FP16 = mybir.dt.float16
AF = mybir.ActivationFunctionType
OP = mybir.AluOpType

H = 128          # hidden
NCORES = 8
BS = 256         # batch per core
BC = 128         # batch per chunk
NCHUNK = 2
BLK = 32         # xq steps per DMA block

# kernel gate order == pytorch order (i, f, g, o): sigma(i,f,g) is one
# contiguous on-chain activation; sigma(o) is separate and off-chain (o is
# first needed only after tanh(c)).
_PERM = np.arange(512)
# g-gate rows doubled so the matmul emits 2*pre_g for the
# tanh(x) = 2*sigmoid(2x)-1 rewrite; cT state = 2c.
_SCALE = np.repeat([1.0, 1.0, 2.0, 1.0], 128)
_SCALE_B = _SCALE


def _build_body(tc, d, NP, NH):
    nc = tc.nc
    NT = NP + NH
    NBLK = (NP + BLK - 1) // BLK

    import contextlib
    with contextlib.ExitStack() as ctx:
        consts = ctx.enter_context(tc.tile_pool(name="consts", bufs=1))
        state = ctx.enter_context(tc.tile_pool(name="state", bufs=1))
        spool = ctx.enter_context(tc.tile_pool(name="sig", bufs=3))
        wpool = ctx.enter_context(tc.tile_pool(name="work", bufs=3))
        xpool = ctx.enter_context(tc.tile_pool(name="xq", bufs=2))
        gpool = ctx.enter_context(tc.tile_pool(name="gates", bufs=2, space="PSUM"))
        ppool = ctx.enter_context(tc.tile_pool(name="ppsum", bufs=1, space="PSUM"))

        # ---- constants to SBUF
        whhT_p = consts.tile([H, 4 * H], FP16, tag="whhT_p")
        whhT_h = consts.tile([H, 4 * H], FP16, tag="whhT_h")
        bp8 = consts.tile([8, H], FP16, tag="bp8")
        bh4 = consts.tile([4, H], FP16, tag="bh4")
        ones4 = consts.tile([4, 4 * BC], FP16, tag="ones4")
        woutZ = consts.tile([H, 2 * H], FP16, tag="woutZ")
        nc.sync.dma_start(out=whhT_p, in_=d["whhT_p"])
        nc.sync.dma_start(out=whhT_h, in_=d["whhT_h"])
        nc.sync.dma_start(out=bp8, in_=d["bp8"])
        nc.sync.dma_start(out=bh4, in_=d["bh4"])
        nc.sync.dma_start(out=ones4, in_=d["ones4"])
        nc.sync.dma_start(out=woutZ, in_=d["woutZ"])

        # ---- state
        hist = []
        cT = []
        for ch in range(NCHUNK):
            hh = state.tile([H, 4 * BC], FP16, tag=f"hist{ch}")
            c = state.tile([H, BC], FP16, tag=f"cT{ch}")
            nc.vector.memset(hh, 0.0)
            nc.vector.memset(c, 0.0)
            hist.append(hh)
            cT.append(c)

        # ---- xq stream (phase P block-diag rhs), double buffered
        xtiles = [[None] * NBLK for _ in range(NCHUNK)]

        def fetch(blk):
            for ch in range(NCHUNK):
                xt = xpool.tile([8, BLK * 4 * BC], FP16, tag=f"xq{ch}",
                                name=f"xq{ch}_{blk}")
                nc.sync.dma_start(out=xt, in_=d["xq"][ch, blk])
                xtiles[ch][blk] = xt

        fetch(0)
        fetch(1)

        s4s = [None, None]
        sos = [None, None]
        pps = [None, None]

        def front(s, ch):
            """input/bias MM + 4 gate MMs + one merged sigmoid."""
            phase_p = s < NP
            gates = gpool.tile([H, 4 * BC], F32, tag=f"g{ch}",
                               name=f"g{ch}_{s}")
            if phase_p:
                blk, sl = divmod(s, BLK)
                rhs = xtiles[ch][blk][:, sl * 4 * BC:(sl + 1) * 4 * BC]
                nc.tensor.matmul(gates, bp8, rhs, start=True, stop=False,
                                 skip_group_check=True)
            else:
                nc.tensor.matmul(gates, bh4, ones4, start=True, stop=False,
                                 skip_group_check=True)
            whh = whhT_p if phase_p else whhT_h
            hprev = hist[ch][:, ((s - 1) % 4) * BC: ((s - 1) % 4 + 1) * BC]
            for j in range(4):
                nc.tensor.matmul(gates[:, j * H:(j + 1) * H],
                                 whh[:, j * H:(j + 1) * H], hprev,
                                 start=False, stop=(j == 3),
                                 skip_group_check=True)
            s4 = spool.tile([H, 3 * BC], FP16, tag=f"s4{ch}",
                            name=f"s4{ch}_{s}")
            nc.scalar.activation(s4, gates[:, 0:3 * H], AF.Sigmoid)
            so = spool.tile([H, BC], FP16, tag=f"so{ch}", name=f"so{ch}_{s}")
            nc.scalar.activation(so, gates[:, 3 * H:4 * H], AF.Sigmoid)
            s4s[ch] = s4
            sos[ch] = so

        def back(s, ch):
            """cell update on DVE + sigma(cT) + hT + batched prediction MM."""
            s4 = s4s[ch]
            t2 = wpool.tile([H, BC], FP16, tag=f"t2{ch}", name=f"t2{ch}_{s}")
            nc.vector.tensor_mul(t2, s4[:, H:2 * H], cT[ch])
            u = wpool.tile([H, BC], FP16, tag=f"u{ch}", name=f"u{ch}_{s}")
            nc.vector.scalar_tensor_tensor(u, s4[:, 2 * H:3 * H], 0.5,
                                           s4[:, 0:H], OP.subtract, OP.mult)
            nc.vector.scalar_tensor_tensor(cT[ch], u, 4.0, t2,
                                           OP.mult, OP.add)
            tc_ = wpool.tile([H, BC], FP16, tag=f"sc{ch}", name=f"sc{ch}_{s}")
            nc.scalar.activation(tc_, cT[ch], AF.Tanh, scale=0.5)
            hslot = hist[ch][:, (s % 4) * BC: (s % 4 + 1) * BC]
            nc.vector.tensor_mul(hslot, tc_, sos[ch])

            # Predictions: every 4 steps, one matmul W_out @ [h0|h1|h2|h3];
            # row placement via shifted zero-padded stationary.
            if s % 4 == 3 or s == NT - 1:
                G = s // 4
                r = G % 32
                n = (s % 4 + 1) * BC
                if r == 0:
                    pps[ch] = ppool.tile([H, 4 * BC], F32, tag=f"pps{ch}",
                                         name=f"pps{ch}_{s}")
                nc.tensor.matmul(pps[ch][:, 0:n],
                                 woutZ[:, H - r: 2 * H - r],
                                 hist[ch][:, 0:n],
                                 start=(r == 0), stop=(r == 31 or s == NT - 1),
                                 skip_group_check=True)
                if r == 31 or s == NT - 1:
                    e = G // 32
                    pc = wpool.tile([32, 4 * BC], F32, tag=f"pc{ch}",
                                    name=f"pc{ch}_{s}")
                    nc.vector.tensor_copy(pc, pps[ch][0:32, :])
                    nc.sync.dma_start(out=d["preds"][e, ch], in_=pc)

        # Software pipeline: full A-step then full B-step per iteration.
        # Each engine's FIFO then alternates A-stage / B-stage, which locks
        # the two chunks half a step out of phase (emitting both fronts
        # together lets the chunks drift in-phase and exposes the full
        # serial chain latency).
        for s in range(NT):
            if s % BLK == BLK // 2:
                nb = s // BLK + 2
                if nb < NBLK:
                    fetch(nb)
            front(s, 0)
            back(s, 0)
            front(s, 1)
            back(s, 1)


@functools.lru_cache(maxsize=2)
def _program(NP, NH):
    nc = bacc.Bacc("TRN2", target_bir_lowering=False, debug=False,
                   num_devices=NCORES)
    NT = NP + NH
    NEP = (NT + 127) // 128
    NBLK = (NP + BLK - 1) // BLK
    d = {
        "whhT_p": nc.dram_tensor("whhT_p", [H, 4 * H], FP16,
                                 kind="ExternalInput").ap(),
        "whhT_h": nc.dram_tensor("whhT_h", [H, 4 * H], FP16,
                                 kind="ExternalInput").ap(),
        "bp8": nc.dram_tensor("bp8", [8, H], FP16, kind="ExternalInput").ap(),
        "bh4": nc.dram_tensor("bh4", [4, H], FP16, kind="ExternalInput").ap(),
        "ones4": nc.dram_tensor("ones4", [4, 4 * BC], FP16,
                                kind="ExternalInput").ap(),
        "woutZ": nc.dram_tensor("woutZ", [H, 2 * H], FP16,
                                kind="ExternalInput").ap(),
        "xq": nc.dram_tensor("xq", [NCHUNK, NBLK, 8, BLK * 4 * BC], FP16,
                             kind="ExternalInput").ap(),
        "preds": nc.dram_tensor("preds", [NEP, NCHUNK, 32, 4 * BC], F32,
                                kind="ExternalOutput").ap(),
    }
    with tile.TileContext(nc) as tc:
        _build_body(tc, d, NP, NH)
    nc.compile()
    return nc


def _host_prep(y_flow, W_ih, W_hh, b_ih, b_hh, W_out, b_out, NP):
    """Build per-core input maps. y_flow: (B, T, 1) f32."""
    f16 = np.float16
    W_ih = np.asarray(W_ih, np.float32)
    W_hh = np.asarray(W_hh, np.float32)
    W_out = np.asarray(W_out, np.float32)
    bias = np.asarray(b_ih, np.float32) + np.asarray(b_hh, np.float32)
    b_out = np.asarray(b_out, np.float32)

    W_eff = W_hh + W_ih @ W_out           # [4H, H] (phase-H feedback fold)
    b_eff = bias + W_ih[:, 0] * b_out[0]

    sc = _SCALE[:, None]
    whhT_p = np.ascontiguousarray((W_hh[_PERM] * sc).T).astype(f16)
    whhT_h = np.ascontiguousarray((W_eff[_PERM] * sc).T).astype(f16)

    wih_s = (W_ih[_PERM, 0] * _SCALE_B).astype(np.float32)
    b_s = (bias[_PERM] * _SCALE_B).astype(np.float32)
    beff_s = (b_eff[_PERM] * _SCALE_B).astype(np.float32)

    bp8 = np.zeros((8, H), np.float32)
    bh4 = np.zeros((4, H), np.float32)
    ones4 = np.zeros((4, 4 * BC), np.float32)
    for j in range(4):
        bp8[2 * j] = wih_s[j * H:(j + 1) * H]
        bp8[2 * j + 1] = b_s[j * H:(j + 1) * H]
        bh4[j] = beff_s[j * H:(j + 1) * H]
        ones4[j, j * BC:(j + 1) * BC] = 1.0

    woutZ = np.zeros((H, 2 * H), np.float32)
    woutZ[:, H] = W_out[0]

    NBLK = (NP + BLK - 1) // BLK
    NPAD = NBLK * BLK
    y = np.asarray(y_flow, np.float32)[:, :, 0]                   # [B, T]
    in_maps = []
    for core in range(NCORES):
        yc = y[core * BS:(core + 1) * BS]                         # [BS, T]
        xq = np.zeros((NCHUNK, NPAD, 8, 4 * BC), np.float32)
        for ch in range(NCHUNK):
            ystep = yc[ch * BC:(ch + 1) * BC, :NP].T              # [NP, BC]
            for j in range(4):
                xq[ch, :NP, 2 * j, j * BC:(j + 1) * BC] = ystep
                xq[ch, :, 2 * j + 1, j * BC:(j + 1) * BC] = 1.0
        # [ch, NBLK, BLK, 8, 512] -> [ch, NBLK, 8, BLK*512]
        xq = xq.reshape(NCHUNK, NBLK, BLK, 8, 4 * BC)
        xq = np.ascontiguousarray(xq.transpose(0, 1, 3, 2, 4))
        xq = xq.reshape(NCHUNK, NBLK, 8, BLK * 4 * BC)
        in_maps.append({
            "whhT_p": whhT_p, "whhT_h": whhT_h,
            "bp8": bp8.astype(f16), "bh4": bh4.astype(f16),
            "ones4": ones4.astype(f16), "woutZ": woutZ.astype(f16),
            "xq": xq.astype(f16),
        })
    return in_maps


def kernel(y_flow, x_dyn, W_ih, W_hh, b_ih, b_hh, W_out, b_out, twin_idx,
           _trace=False):
    twin = int(twin_idx)
    assert twin == 256, f"kernel hardcodes twin_idx=256, got {twin}"
    B, T, _ = y_flow.shape
    assert (B, T) == (2048, 512)
    NP, NH = twin - 1, T - twin
    NT = NP + NH

    nc = _program(NP, NH)
    in_maps = _host_prep(y_flow, W_ih, W_hh, b_ih, b_hh, W_out, b_out, NP)
    res = run_bass_kernel_spmd(nc, in_maps, core_ids=list(range(NCORES)),
                               trace=_trace)

    b_out = np.asarray(b_out, np.float32)
    out = np.empty((B, NT, 1), np.float32)
    for core in range(NCORES):
        p = np.asarray(res.results[core]["preds"], np.float32)
        nep = p.shape[0]
        a = p.reshape(nep, NCHUNK, 32, 4, BC)      # [e, ch, r, j, b]
        for ch in range(NCHUNK):
            blk = a[:, ch].transpose(3, 0, 1, 2).reshape(BC, -1)[:, :NT]
            out[core * BS + ch * BC: core * BS + (ch + 1) * BC, :, 0] = \
                blk + b_out[0]
    if _trace:
        kernel._last_results = res
    return out


# revision 27
# speedup vs baseline: 1.7976x; 1.0312x over previous
"""Bass/Trainium2 kernel for nn_BaselineLSTM (B=2048, T=512, H=128, twin=256).

Strategy (v2):
  - Data-parallel: batch 2048 -> 8 cores x 256; each core runs 2 interleaved
    chunks of 128 batch (pipelining hides per-step cross-engine latency).
  - State kept transposed: h/c = [H=128 partitions, batch free]; state
    variables are scaled: hT = h/2, cT = 2c, so that every tanh can be
    computed as a sigmoid and all fix-up constants fold into weights:
      tanh(x) = 2*sigmoid(2x) - 1.
  - ONE sigmoid ACT per chunk-step covers all four gates [i|f|o|g]: the
    g-block rows of the stationary weights are pre-scaled so the matmul
    emits 2*pre_g there; a second small sigmoid covers sigma(cT)=sigma(2c).
  - Input + bias enter via ONE K=8 (phase P) / K=4 (phase H) matmul with a
    block-diagonal rhs (phase P rhs streamed from DRAM, phase H rhs static),
    accumulated into the gates PSUM bank before the 4 recurrent matmuls.
  - Cell update on DVE only (gpsimd is pathologically slow for elementwise):
      t2 = sf*cT;  u = (s2g-0.5)*si;  cT = 4u + t2       (scalar_tensor_tensor)
      hT = (sigma(cT)-0.5)*so                             (scalar_tensor_tensor)
  - fp16 everywhere on-chip (not bf16): the 2*sigmoid(2x)-1 rewrite loses
    absolute precision near 0.5 in bf16; fp16's 10 mantissa bits restore it,
    and fp16 keeps the DVE 2x/4x packed modes.
  - Predictions p_t = (2*W_out) hT_t (+ b_out on host): hT kept in a 4-slot
    ring; one shifted-stationary matmul per 4 steps accumulates 128 steps
    into one PSUM bank, flushed to DRAM per 128-step epoch.
"""

import functools

import numpy as np

import concourse.bacc as bacc
import concourse.tile as tile
from concourse import mybir
from concourse.bass_utils import run_bass_kernel_spmd

F32 = mybir.dt.float32
FP16 = mybir.dt.float16
AF = mybir.ActivationFunctionType
OP = mybir.AluOpType

H = 128          # hidden
NCORES = 8
BS = 256         # batch per core
BC = 128         # batch per chunk
NCHUNK = 2
BLK = 32         # xq steps per DMA block

# kernel gate order == pytorch order (i, f, g, o): sigma(i,f,g) is one
# contiguous on-chain activation; sigma(o) is separate and off-chain (o is
# first needed only after tanh(c)).
_PERM = np.arange(512)
# g-gate rows doubled so the matmul emits 2*pre_g for the
# tanh(x) = 2*sigmoid(2x)-1 rewrite; cT state = 2c.
_SCALE = np.repeat([1.0, 1.0, 2.0, 1.0], 128)
_SCALE_B = _SCALE


def _build_body(tc, d, NP, NH):
    nc = tc.nc
    NT = NP + NH
    NBLK = (NP + BLK - 1) // BLK

    import contextlib
    with contextlib.ExitStack() as ctx:
        consts = ctx.enter_context(tc.tile_pool(name="consts", bufs=1))
        state = ctx.enter_context(tc.tile_pool(name="state", bufs=1))
        spool = ctx.enter_context(tc.tile_pool(name="sig", bufs=3))
        wpool = ctx.enter_context(tc.tile_pool(name="work", bufs=3))
        xpool = ctx.enter_context(tc.tile_pool(name="xq", bufs=2))
        gpool = ctx.enter_context(tc.tile_pool(name="gates", bufs=2, space="PSUM"))
        ppool = ctx.enter_context(tc.tile_pool(name="ppsum", bufs=1, space="PSUM"))

        # ---- constants to SBUF
        whhT_p = consts.tile([H, 4 * H], FP16, tag="whhT_p")
        whhT_h = consts.tile([H, 4 * H], FP16, tag="whhT_h")
        bp8 = consts.tile([8, H], FP16, tag="bp8")
        bh4 = consts.tile([4, H], FP16, tag="bh4")
        ones4 = consts.tile([4, 4 * BC], FP16, tag="ones4")
        woutZ = consts.tile([H, 2 * H], FP16, tag="woutZ")
        nc.sync.dma_start(out=whhT_p, in_=d["whhT_p"])
        nc.sync.dma_start(out=whhT_h, in_=d["whhT_h"])
        nc.sync.dma_start(out=bp8, in_=d["bp8"])
        nc.sync.dma_start(out=bh4, in_=d["bh4"])
        nc.sync.dma_start(out=ones4, in_=d["ones4"])
        nc.sync.dma_start(out=woutZ, in_=d["woutZ"])

        # ---- state
        hist = []
        cT = []
        for ch in range(NCHUNK):
            hh = state.tile([H, 8 * BC], FP16, tag=f"hist{ch}")
            c = state.tile([H, BC], FP16, tag=f"cT{ch}")
            nc.vector.memset(hh, 0.0)
            nc.vector.memset(c, 0.0)
            hist.append(hh)
            cT.append(c)

        # ---- xq stream (phase P block-diag rhs), double buffered
        xtiles = [[None] * NBLK for _ in range(NCHUNK)]

        def fetch(blk):
            for ch in range(NCHUNK):
                xt = xpool.tile([8, BLK * 4 * BC], FP16, tag=f"xq{ch}",
                                name=f"xq{ch}_{blk}")
                nc.sync.dma_start(out=xt, in_=d["xq"][ch, blk])
                xtiles[ch][blk] = xt

        fetch(0)
        fetch(1)

        s4s = [None, None]
        sos = [None, None]
        pps = [None, None]

        def front(s, ch):
            """input/bias MM + 4 gate MMs + one merged sigmoid."""
            phase_p = s < NP
            gates = gpool.tile([H, 4 * BC], F32, tag=f"g{ch}",
                               name=f"g{ch}_{s}")
            if phase_p:
                blk, sl = divmod(s, BLK)
                rhs = xtiles[ch][blk][:, sl * 4 * BC:(sl + 1) * 4 * BC]
                nc.tensor.matmul(gates, bp8, rhs, start=True, stop=False,
                                 skip_group_check=True)
            else:
                nc.tensor.matmul(gates, bh4, ones4, start=True, stop=False,
                                 skip_group_check=True)
            whh = whhT_p if phase_p else whhT_h
            hprev = hist[ch][:, ((s - 1) % 8) * BC: ((s - 1) % 8 + 1) * BC]
            for j in range(4):
                nc.tensor.matmul(gates[:, j * H:(j + 1) * H],
                                 whh[:, j * H:(j + 1) * H], hprev,
                                 start=False, stop=(j == 3),
                                 skip_group_check=True)
            s4 = spool.tile([H, 3 * BC], FP16, tag=f"s4{ch}",
                            name=f"s4{ch}_{s}")
            nc.scalar.activation(s4, gates[:, 0:3 * H], AF.Sigmoid)
            so = spool.tile([H, BC], FP16, tag=f"so{ch}", name=f"so{ch}_{s}")
            nc.scalar.activation(so, gates[:, 3 * H:4 * H], AF.Sigmoid)
            s4s[ch] = s4
            sos[ch] = so

        def back(s, ch):
            """cell update on DVE + sigma(cT) + hT + batched prediction MM."""
            s4 = s4s[ch]
            t2 = wpool.tile([H, BC], FP16, tag=f"t2{ch}", name=f"t2{ch}_{s}")
            nc.vector.tensor_mul(t2, s4[:, H:2 * H], cT[ch])
            u = wpool.tile([H, BC], FP16, tag=f"u{ch}", name=f"u{ch}_{s}")
            nc.vector.scalar_tensor_tensor(u, s4[:, 2 * H:3 * H], 0.5,
                                           s4[:, 0:H], OP.subtract, OP.mult)
            nc.vector.scalar_tensor_tensor(cT[ch], u, 4.0, t2,
                                           OP.mult, OP.add)
            tc_ = wpool.tile([H, BC], FP16, tag=f"sc{ch}", name=f"sc{ch}_{s}")
            nc.scalar.activation(tc_, cT[ch], AF.Tanh, scale=0.5)
            hslot = hist[ch][:, (s % 8) * BC: (s % 8 + 1) * BC]
            nc.vector.tensor_mul(hslot, tc_, sos[ch])

            # Predictions for group G (steps 4G..4G+3) are emitted one step
            # AFTER the group completes (s = 4G+4): by then every hist slot
            # the matmul reads is already written, so the in-order PE queue
            # never stalls on it (the 8-slot ring gives a 4-step reuse gap).
            if s % 4 == 0 and s >= 4:
                emit_pred(s // 4 - 1, ch)

        def emit_pred(G, ch):
            NG = (NT + 3) // 4
            r = G % 32
            n = min(NT - 4 * G, 4) * BC
            base = (G % 2) * 4 * BC
            if r == 0:
                pps[ch] = ppool.tile([H, 4 * BC], F32, tag=f"pps{ch}",
                                     name=f"pps{ch}_{G}")
            nc.tensor.matmul(pps[ch][:, 0:n],
                             woutZ[:, H - r: 2 * H - r],
                             hist[ch][:, base: base + n],
                             start=(r == 0), stop=(r == 31 or G == NG - 1),
                             skip_group_check=True)
            if r == 31 or G == NG - 1:
                e = G // 32
                pc = wpool.tile([32, 4 * BC], F32, tag=f"pc{ch}",
                                name=f"pc{ch}_{G}")
                nc.vector.tensor_copy(pc, pps[ch][0:32, :])
                nc.sync.dma_start(out=d["preds"][e, ch], in_=pc)

        # Software pipeline: full A-step then full B-step per iteration.
        # Each engine's FIFO then alternates A-stage / B-stage, which locks
        # the two chunks half a step out of phase (emitting both fronts
        # together lets the chunks drift in-phase and exposes the full
        # serial chain latency).
        for s in range(NT):
            if s % BLK == BLK // 2:
                nb = s // BLK + 2
                if nb < NBLK:
                    fetch(nb)
            front(s, 0)
            back(s, 0)
            front(s, 1)
            back(s, 1)

        # final prediction group(s) not yet emitted by the loop
        NG = (NT + 3) // 4
        emit_pred(NG - 1, 0)
        emit_pred(NG - 1, 1)


@functools.lru_cache(maxsize=2)
def _program(NP, NH):
    nc = bacc.Bacc("TRN2", target_bir_lowering=False, debug=False,
                   num_devices=NCORES)
    NT = NP + NH
    NEP = (NT + 127) // 128
    NBLK = (NP + BLK - 1) // BLK
    d = {
        "whhT_p": nc.dram_tensor("whhT_p", [H, 4 * H], FP16,
                                 kind="ExternalInput").ap(),
        "whhT_h": nc.dram_tensor("whhT_h", [H, 4 * H], FP16,
                                 kind="ExternalInput").ap(),
        "bp8": nc.dram_tensor("bp8", [8, H], FP16, kind="ExternalInput").ap(),
        "bh4": nc.dram_tensor("bh4", [4, H], FP16, kind="ExternalInput").ap(),
        "ones4": nc.dram_tensor("ones4", [4, 4 * BC], FP16,
                                kind="ExternalInput").ap(),
        "woutZ": nc.dram_tensor("woutZ", [H, 2 * H], FP16,
                                kind="ExternalInput").ap(),
        "xq": nc.dram_tensor("xq", [NCHUNK, NBLK, 8, BLK * 4 * BC], FP16,
                             kind="ExternalInput").ap(),
        "preds": nc.dram_tensor("preds", [NEP, NCHUNK, 32, 4 * BC], F32,
                                kind="ExternalOutput").ap(),
    }
    with tile.TileContext(nc) as tc:
        _build_body(tc, d, NP, NH)
    nc.compile()
    return nc


def _host_prep(y_flow, W_ih, W_hh, b_ih, b_hh, W_out, b_out, NP):
    """Build per-core input maps. y_flow: (B, T, 1) f32."""
    f16 = np.float16
    W_ih = np.asarray(W_ih, np.float32)
    W_hh = np.asarray(W_hh, np.float32)
    W_out = np.asarray(W_out, np.float32)
    bias = np.asarray(b_ih, np.float32) + np.asarray(b_hh, np.float32)
    b_out = np.asarray(b_out, np.float32)

    W_eff = W_hh + W_ih @ W_out           # [4H, H] (phase-H feedback fold)
    b_eff = bias + W_ih[:, 0] * b_out[0]

    sc = _SCALE[:, None]
    whhT_p = np.ascontiguousarray((W_hh[_PERM] * sc).T).astype(f16)
    whhT_h = np.ascontiguousarray((W_eff[_PERM] * sc).T).astype(f16)

    wih_s = (W_ih[_PERM, 0] * _SCALE_B).astype(np.float32)
    b_s = (bias[_PERM] * _SCALE_B).astype(np.float32)
    beff_s = (b_eff[_PERM] * _SCALE_B).astype(np.float32)

    bp8 = np.zeros((8, H), np.float32)
    bh4 = np.zeros((4, H), np.float32)
    ones4 = np.zeros((4, 4 * BC), np.float32)
    for j in range(4):
        bp8[2 * j] = wih_s[j * H:(j + 1) * H]
        bp8[2 * j + 1] = b_s[j * H:(j + 1) * H]
        bh4[j] = beff_s[j * H:(j + 1) * H]
        ones4[j, j * BC:(j + 1) * BC] = 1.0

    woutZ = np.zeros((H, 2 * H), np.float32)
    woutZ[:, H] = W_out[0]

    NBLK = (NP + BLK - 1) // BLK
    NPAD = NBLK * BLK
    y = np.asarray(y_flow, np.float32)[:, :, 0]                   # [B, T]
    in_maps = []
    for core in range(NCORES):
        yc = y[core * BS:(core + 1) * BS]                         # [BS, T]
        xq = np.zeros((NCHUNK, NPAD, 8, 4 * BC), np.float32)
        for ch in range(NCHUNK):
            ystep = yc[ch * BC:(ch + 1) * BC, :NP].T              # [NP, BC]
            for j in range(4):
                xq[ch, :NP, 2 * j, j * BC:(j + 1) * BC] = ystep
                xq[ch, :, 2 * j + 1, j * BC:(j + 1) * BC] = 1.0
        # [ch, NBLK, BLK, 8, 512] -> [ch, NBLK, 8, BLK*512]
        xq = xq.reshape(NCHUNK, NBLK, BLK, 8, 4 * BC)
        xq = np.ascontiguousarray(xq.transpose(0, 1, 3, 2, 4))
        xq = xq.reshape(NCHUNK, NBLK, 8, BLK * 4 * BC)
        in_maps.append({
            "whhT_p": whhT_p, "whhT_h": whhT_h,
            "bp8": bp8.astype(f16), "bh4": bh4.astype(f16),
            "ones4": ones4.astype(f16), "woutZ": woutZ.astype(f16),
            "xq": xq.astype(f16),
        })
    return in_maps


def kernel(y_flow, x_dyn, W_ih, W_hh, b_ih, b_hh, W_out, b_out, twin_idx,
           _trace=False):
    twin = int(twin_idx)
    assert twin == 256, f"kernel hardcodes twin_idx=256, got {twin}"
    B, T, _ = y_flow.shape
    assert (B, T) == (2048, 512)
    NP, NH = twin - 1, T - twin
    NT = NP + NH

    nc = _program(NP, NH)
    in_maps = _host_prep(y_flow, W_ih, W_hh, b_ih, b_hh, W_out, b_out, NP)
    res = run_bass_kernel_spmd(nc, in_maps, core_ids=list(range(NCORES)),
                               trace=_trace)

    b_out = np.asarray(b_out, np.float32)
    out = np.empty((B, NT, 1), np.float32)
    for core in range(NCORES):
        p = np.asarray(res.results[core]["preds"], np.float32)
        nep = p.shape[0]
        a = p.reshape(nep, NCHUNK, 32, 4, BC)      # [e, ch, r, j, b]
        for ch in range(NCHUNK):
            blk = a[:, ch].transpose(3, 0, 1, 2).reshape(BC, -1)[:, :NT]
            out[core * BS + ch * BC: core * BS + (ch + 1) * BC, :, 0] = \
                blk + b_out[0]
    if _trace:
        kernel._last_results = res
    return out
